# revision 51
# baseline (speedup 1.0000x reference)
"""Trainium2 Bass kernel for nn_Attention (LN -> QKV -> MHA -> out-proj).

Sharding: 8 cores = 4 batches x 2 head-groups. Core c handles batch c//2,
heads (c%2)*8 .. (c%2)*8+8 (tensor-parallel split of w_qkv columns / w_out
rows). Each core emits a partial [2048, 1024] output; host sums the two
partials per batch.

Host-side precompute: LN gamma is folded into w_qkv rows (exact); beta
becomes a rank-1 bias row beta@w_qkv added on device only when nonzero.
All per-core inputs ship as ONE packed f32 dram tensor ("blob") because
per-exec dispatch overhead in this runtime grows ~60us per bound IO
tensor: [x (2048x1024 f32) | w_qkv' (1024x1536 bf16) | w_out (512x1024
bf16) | bias (1536 bf16)], bf16 regions bitcast from f32 words.

Device algorithm (per core), matmuls bf16 with f32 PSUM accumulation.
Schedule notes: GPSIMD cannot read PSUM on real TRN2 (CoreSim allows it),
so PSUM->SBUF copies go to ACT in phase 1 (idle there) and DVE elsewhere;
engine queues are in-order, so background work is placed where its
dependencies are already drained.
  1. Phase 1 per token tile: LayerNorm (f32 stats via bn_stats on DVE),
     cast bf16, PE-transpose to xnT [1024, 2048] (features on
     partitions) with ACT copying PSUM->SBUF; v = xn @ wv one group
     behind (ones column appended for the softmax denominator); all
     k-side projections kT = wk^T @ xnT plus the qp=0 half of qT; the
     whole first attention block (j=0, qp=0) weaves in as v/k tiles
     complete, so ACT's exp pipeline starts inside phase 1.
  2. Attention blocks (head pair j, query half qp): per kv tile t,
     scoresT = kT^T qT and exp(scale*scoresT) on ACT (no max
     subtraction -- scores are O(5) for these inputs) emit first, then
     one background unit, then the PREVIOUS tile's PV matmuls -- PE
     bursts from background work land ahead of a deferrable PV instead
     of stalling the exp feed (ACT is the in-block bottleneck at
     ~2.08us/tile vs PE ~1.7us). PV accumulates [out^T; denom] in PSUM.
     At block end only reciprocal (DVE) + raw copy (DVE) run, freeing
     PSUM for the next block; the normalize finish (ones-matmul
     broadcast + multiply into outn) defers into the next block's
     background slots (t0+2, t0+4, ...) where its chain has drained.
     Background slots also carry the q-side n=2,3 projections (only
     needed from qp=1 on) and, during qp=1, the out-projection of the
     qp=0 token half.
  3. out = sum_j outn[j]^T @ w_out[j], K=128 PSUM accumulation; DVE
     copies PSUM->SBUF; the qp=1 token half drains in a short tail.
"""

import os
import sys

sys.path.insert(0, "/opt/trn_rl_repo")
os.environ.setdefault("MYCRO_LOCAL_CACHE", "1")

import numpy as np
import ml_dtypes

N_TOK = 2048
DIM = 1024
HPC = 8          # heads per core
DH = 64          # head dim
INNER_C = HPC * DH  # 512 per-core inner width
NT = N_TOK // 128   # 16 token tiles
KF = DIM // 128     # 8 feature tiles
SCALE = DH ** -0.5

# blob layout (f32 word offsets)
X_WORDS = N_TOK * DIM                      # 2,097,152
WQ_WORDS = DIM * 3 * INNER_C // 2          # 786,432 (bf16 pairs)
WO_WORDS = INNER_C * DIM // 2              # 262,144
B_WORDS = 3 * INNER_C // 2                 # 768
X_OFF = 0
WQ_OFF = X_OFF + X_WORDS
WO_OFF = WQ_OFF + WQ_WORDS
B_OFF = WO_OFF + WO_WORDS
BLOB_WORDS = B_OFF + B_WORDS

_cache = {}


def _build_nc(has_bias):
    import concourse.bass as bass
    import concourse.mybir as mybir
    import concourse.tile as tile
    from concourse import bacc
    from concourse.masks import make_identity
    from contextlib import ExitStack

    f32 = mybir.dt.float32
    bf16 = mybir.dt.bfloat16
    nc = bacc.Bacc(None, target_bir_lowering=False)

    blob_d = nc.dram_tensor("blob", [BLOB_WORDS], f32, kind="ExternalInput")
    out_d = nc.dram_tensor("out", [N_TOK, DIM], f32, kind="ExternalOutput")

    def wq_slice(kc):
        w = blob_d[WQ_OFF + kc * 128 * (3 * INNER_C // 2):
                   WQ_OFF + (kc + 1) * 128 * (3 * INNER_C // 2)]
        return w.bitcast(bf16).rearrange("(p f) -> p f", p=128)

    def wo_slice(j):
        w = blob_d[WO_OFF + j * 128 * (DIM // 2):
                   WO_OFF + (j + 1) * 128 * (DIM // 2)]
        return w.bitcast(bf16).rearrange("(p f) -> p f", p=128)

    def x_slice(t):
        return blob_d[t * 128 * DIM:(t + 1) * 128 * DIM].rearrange(
            "(p f) -> p f", p=128)

    with tile.TileContext(nc) as tc, ExitStack() as ctx:
        consts = ctx.enter_context(tc.tile_pool(name="consts", bufs=1))
        weights = ctx.enter_context(tc.tile_pool(name="weights", bufs=1))
        persist = ctx.enter_context(tc.tile_pool(name="persist", bufs=1))
        work = ctx.enter_context(tc.tile_pool(name="work", bufs=3))
        nrm = ctx.enter_context(tc.tile_pool(name="nrm", bufs=1))
        stats = ctx.enter_context(tc.tile_pool(name="stats", bufs=4))
        mm_ps = ctx.enter_context(tc.tile_pool(name="mm_ps", bufs=2, space="PSUM"))
        pv_ps = ctx.enter_context(tc.tile_pool(name="pv_ps", bufs=2, space="PSUM"))

        ident = consts.tile([128, 128], bf16, tag="ident")
        make_identity(nc, ident)
        eps_t = consts.tile([128, 1], f32, tag="eps")
        nc.vector.memset(eps_t, 1e-5)
        ones_t = consts.tile([1, 64], bf16, tag="ones_t")
        nc.vector.memset(ones_t, 1.0)
        if has_bias:
            bias_sb = consts.tile([1, 3 * INNER_C], bf16, tag="bias")
            nc.sync.dma_start(
                out=bias_sb,
                in_=blob_d[B_OFF:B_OFF + B_WORDS].bitcast(bf16).rearrange(
                    "(p f) -> p f", p=1))
            ones_row = consts.tile([1, 512], bf16, tag="ones_row")
            nc.vector.memset(ones_row, 1.0)

        wo_sb = [weights.tile([128, DIM], bf16, tag=f"wo{j}", name=f"wo{j}")
                 for j in range(4)]

        qkT = [persist.tile([128, N_TOK], bf16, tag=f"qkT{m}", name=f"qkT{m}")
               for m in range(KF)]
        outn = [persist.tile([128, N_TOK], bf16, tag=f"outn{j}", name=f"outn{j}")
                for j in range(4)]
        v_aug = [persist.tile([128, HPC, DH + 1], bf16, tag=f"vaug{t}",
                              name=f"vaug{t}") for t in range(NT)]

        def out_proj(t):
            ps_o = mm_ps.tile([128, 1024], f32, tag="mm", name="ops")
            for c in range(2):
                for j in range(4):
                    nc.tensor.matmul(
                        ps_o[:, c * 512:(c + 1) * 512],
                        lhsT=outn[j][:, t * 128:(t + 1) * 128],
                        rhs=wo_sb[j][:, c * 512:(c + 1) * 512],
                        start=(j == 0), stop=(j == 3),
                    )
            osb = work.tile([128, DIM], f32, tag="osb", bufs=3)
            nc.vector.tensor_copy(out=osb, in_=ps_o)
            nc.sync.dma_start(out=out_d[t * 128:(t + 1) * 128, :], in_=osb)

        with tc.tile_pool(name="qkvw", bufs=1) as qkvw:
            wq_sb = [qkvw.tile([128, 3 * INNER_C], bf16, tag=f"wq{kc}",
                               name=f"wq{kc}") for kc in range(KF)]
            xnT = [qkvw.tile([128, N_TOK], bf16, tag=f"xnT{f}", name=f"xnT{f}")
                   for f in range(KF)]

            def add_bias(ps_view, col_lo, n):
                # ps_view[m, q] += bias[col_lo + m] for all q (rank-1).
                nc.tensor.matmul(
                    ps_view,
                    lhsT=bias_sb[:, col_lo:col_lo + 128],
                    rhs=ones_row[:, 0:n],
                    start=False, stop=True,
                )

            def v_group(t):
                vt = v_aug[t]
                nc.vector.memset(vt[:, :, DH:DH + 1], 1.0)
                ps = mm_ps.tile([128, 1024], f32, tag="mm", name="vps")
                for kc in range(KF):
                    nc.tensor.matmul(
                        ps[:, 0:512],
                        lhsT=xnT[kc][:, t * 128:(t + 1) * 128],
                        rhs=wq_sb[kc][:, 2 * INNER_C:3 * INNER_C],
                        start=(kc == 0),
                        stop=(kc == KF - 1 and not has_bias),
                    )
                if has_bias:
                    # v rows are tokens: ps[tok, m] += 1[tok] * bias_v[m]
                    nc.tensor.matmul(
                        ps[:, 0:512],
                        lhsT=ones_row[:, 0:128],
                        rhs=bias_sb[:, 2 * INNER_C:3 * INNER_C],
                        start=False, stop=True,
                    )
                nc.vector.tensor_copy(
                    out=vt[:, :, 0:DH],
                    in_=ps[:, 0:512].rearrange("p (h d) -> p h d", h=HPC),
                )

            def qk_group(m, n):
                ps = mm_ps.tile([128, 1024], f32, tag="mm", name="qkps")
                for kc in range(KF):
                    nc.tensor.matmul(
                        ps[:, 0:512],
                        lhsT=wq_sb[kc][:, m * 128:(m + 1) * 128],
                        rhs=xnT[kc][:, n * 512:(n + 1) * 512],
                        start=(kc == 0),
                        stop=(kc == KF - 1 and not has_bias),
                    )
                if has_bias:
                    add_bias(ps[:, 0:512], m * 128, 512)
                nc.scalar.copy(
                    out=qkT[m][:, n * 512:(n + 1) * 512], in_=ps[:, 0:512])

            class AttBlock:
                """Head pair j (heads 2j at partitions 0:64, 2j+1 at
                64:128), query half qp. step(t) consumes kv tile t;
                end() frees PSUM (reciprocal + raw copy) and returns two
                deferred finish closures (ones-matmul broadcast +
                normalize multiply into outn)."""

                def __init__(self, j, qp):
                    self.j, self.qp = j, qp
                    self.qlo = qp * 1024
                    self.pending = []
                    self.ps_pv = [pv_ps.tile([65, 1024], f32, tag="pv",
                                             name="pspv") for _ in range(2)]

                def step(self, t, bg=None, flush=True):
                    # scores+exp for t first, bg burst next, then the
                    # OLDEST pending step's PV: bg PE bursts land in front
                    # of a deferrable PV instead of delaying the exp feed.
                    # flush=False accumulates PVs (used to pre-run a
                    # block's exp feed while another still holds PSUM).
                    j, qlo = self.j, self.qlo
                    ets = []
                    for r in range(2):
                        q_ap = qkT[j][r * 64:(r + 1) * 64, :]
                        k_ap = qkT[4 + j][r * 64:(r + 1) * 64, :]
                        ps_s = mm_ps.tile([128, 1024], f32, tag="mm")
                        for half in range(2):
                            q0 = qlo + half * 512
                            nc.tensor.matmul(
                                ps_s[:, half * 512:(half + 1) * 512],
                                lhsT=k_ap[:, t * 128:(t + 1) * 128],
                                rhs=q_ap[:, q0:q0 + 512],
                                start=True, stop=True,
                            )
                        et = work.tile([128, 1024], bf16, tag="et", bufs=6)
                        nc.scalar.activation(
                            out=et, in_=ps_s[:, 0:1024],
                            func=mybir.ActivationFunctionType.Exp,
                            scale=SCALE,
                        )
                        ets.append(et)
                    if bg is not None:
                        bg()
                    if flush:
                        self._flush_pv()
                    self.pending.append((t, ets))

                def _flush_pv(self):
                    if not self.pending:
                        return
                    t, ets = self.pending.pop(0)
                    j = self.j
                    for r in range(2):
                        for c in range(2):
                            nc.tensor.matmul(
                                self.ps_pv[r][:, c * 512:(c + 1) * 512],
                                lhsT=v_aug[t][:, 2 * j + r, :],
                                rhs=ets[r][:, c * 512:(c + 1) * 512],
                                start=(t == 0), stop=(t == NT - 1),
                            )

                def end(self):
                    while self.pending:
                        self._flush_pv()
                    j, qlo = self.j, self.qlo
                    fins = []
                    for r in range(2):
                        rc = nrm.tile([1, 1024], f32, tag="rc", bufs=2)
                        nc.vector.reciprocal(out=rc, in_=self.ps_pv[r][64:65, :])
                        un = nrm.tile([64, 1024], bf16, tag="un", bufs=2)
                        nc.vector.tensor_copy(out=un, in_=self.ps_pv[r][0:64, :])

                        def fin(r=r, rc=rc, un=un):
                            rcb = nrm.tile([1, 1024], bf16, tag="rcb", bufs=2)
                            nc.gpsimd.tensor_copy(out=rcb, in_=rc)
                            for c in range(2):
                                bcps = mm_ps.tile([64, 512], f32, tag="mm",
                                                  name="bcps")
                                nc.tensor.matmul(
                                    bcps, lhsT=ones_t,
                                    rhs=rcb[:, c * 512:(c + 1) * 512],
                                    start=True, stop=True,
                                )
                                nc.vector.tensor_mul(
                                    out=outn[j][r * 64:(r + 1) * 64,
                                                qlo + c * 512:
                                                qlo + (c + 1) * 512],
                                    in0=un[0:64, c * 512:(c + 1) * 512],
                                    in1=bcps,
                                )
                        fins.append(fin)
                    return fins

            def run_block(blk, t0, bg, every=2):
                # bg closures fire one per slot at t = t0+2, t0+2+every,
                # ...: starting two steps in, the previous block's recip/
                # copy chain (which the first fin unit waits on) has
                # drained, so the fin's ones-matmul doesn't stall the
                # in-order PE queue.
                for t in range(t0, NT):
                    idx = (t - t0 - 2) // every
                    use = (bg[idx] if (t - t0) >= 2 and (t - t0 - 2) % every == 0
                           and idx < len(bg) else None)
                    blk.step(t, bg=use)
                return blk.end()

            # ------------ Phase 1: LayerNorm + transpose + qkv ------------
            with tc.tile_pool(name="ln", bufs=1) as lnp:
                for g in range(4):
                    for tt in range(4):
                        t = g * 4 + tt
                        xt = work.tile([128, DIM], f32, tag="xt", bufs=3)
                        nc.sync.dma_start(out=xt, in_=x_slice(t))
                        st = stats.tile([128, 2, 6], f32, tag="bn")
                        xr = xt.rearrange("p (s d) -> p s d", s=2)
                        for s in range(2):
                            nc.vector.bn_stats(out=st[:, s, :], in_=xr[:, s, :])
                        mv = stats.tile([128, 2], f32, tag="mv")
                        nc.vector.bn_aggr(out=mv, in_=st)
                        rsig = stats.tile([128, 1], f32, tag="rsig")
                        nc.scalar.activation(
                            out=rsig, in_=mv[:, 1:2],
                            func=mybir.ActivationFunctionType.Sqrt,
                            bias=eps_t, scale=1.0,
                        )
                        nc.vector.reciprocal(out=rsig, in_=rsig)
                        xn = lnp.tile([128, DIM], bf16, tag=f"xn{tt}",
                                      name=f"xn{tt}", bufs=2)
                        nc.vector.tensor_scalar(
                            out=xn, in0=xt, scalar1=mv[:, 0:1], scalar2=rsig,
                            op0=mybir.AluOpType.subtract,
                            op1=mybir.AluOpType.mult,
                        )
                        # PE transpose into one PSUM tile; ACT (idle in
                        # phase 1) copies it out to xnT. v_group runs one
                        # group behind so w_qkv (loaded after the first x
                        # tiles) is resident when v_group(0) issues.
                        trps = mm_ps.tile([128, KF, 128], bf16, tag="mm",
                                          name="trps")
                        for f in range(KF):
                            nc.tensor.transpose(
                                out=trps[:, f, :],
                                in_=xn[:, f * 128:(f + 1) * 128],
                                identity=ident,
                            )
                        for f in range(KF):
                            nc.scalar.copy(
                                out=xnT[f][:, t * 128:(t + 1) * 128],
                                in_=trps[:, f, :],
                            )
                        if t >= 4:
                            v_group(t - 4)
                    if g == 0:
                        for kc in range(KF):
                            nc.sync.dma_start(out=wq_sb[kc], in_=wq_slice(kc))
                    if g == 1:
                        for j in range(4):
                            nc.sync.dma_start(out=wo_sb[j], in_=wo_slice(j))
                    # Chunks n=0,1 of everything build in phase 1, plus
                    # m=4 (head pair 0's k side, read by the woven att0 at
                    # t=8..15). Remaining n=2,3 chunks ride the attention
                    # bg slots: each block's own k-side chunk fires two
                    # steps before its first kv reader; q-side chunks are
                    # only read by qp=1 blocks.
                    for m in range(KF) if g < 2 else (4,):
                        qk_group(m, g)
                    if g == 3:
                        for t4 in range(NT - 4, NT):
                            v_group(t4)
                    # weave the first attention block's kv steps into
                    # phase 1 as their q/k/v dependencies complete (kv tile
                    # t needs v_aug[t] and kT chunk t//4 from group t//4;
                    # its q columns 0:1024 need groups 0-1).
                    if g == 2:
                        att0 = AttBlock(0, 0)
                        for t in range(8):
                            att0.step(t)
                    elif g == 3:
                        for t in range(8, NT):
                            att0.step(t)

            # block order: (j,qp=0) j=0..3 then (j,qp=1); each block carries
            # the previous block's deferred normalize finish, the deferred
            # q-side n=2,3 projections, and (during qp=1) the out-proj of
            # the qp=0 token half.
            def qk_unit(m, n):
                return lambda: qk_group(m, n)

            fins = att0.end()
            for j in range(1, 4):
                # order matters: this block's k-side chunk n must fire
                # before its first reader at kv step t=4n.
                extra = [qk_unit(4 + j, 2), qk_unit(j - 1, 2),
                         qk_unit(4 + j, 3), qk_unit(j - 1, 3)]
                fins = run_block(AttBlock(j, 0), 0, fins + extra)
            for j in range(4):
                extra = [qk_unit(3, 2 + j)] if j < 2 else []
                ops = [(lambda t=2 * j + tt: out_proj(t)) for tt in range(2)]
                fins = run_block(AttBlock(j, 1), 0, fins + extra + ops)

        # ------------ tail: deferred finish + remaining out-projection ----
        for fin in fins:
            fin()
        for t in range(8, NT):
            out_proj(t)

    nc.compile()
    return nc


def get_nc(has_bias=False):
    key = ("nc", has_bias)
    if key not in _cache:
        _cache[key] = _build_nc(has_bias)
    return _cache[key]


def _pack_bf16(a):
    """bf16 array -> f32 word view (pairs little-endian)."""
    b = np.ascontiguousarray(a.astype(ml_dtypes.bfloat16))
    return b.reshape(-1).view(np.float32)


def shard_inputs(x, ln_gamma, ln_beta, w_qkv, w_out):
    """Returns (per-core input maps, has_bias) for 8 cores."""
    x = np.asarray(x, np.float32)
    g = np.asarray(ln_gamma, np.float32)
    b = np.asarray(ln_beta, np.float32)
    w0 = np.asarray(w_qkv, np.float32)
    bias_full = b @ w0                 # beta through the projection
    w_qkv = w0 * g[:, None]            # fold LN gamma into rows
    w_out = np.asarray(w_out, np.float32)
    in_maps = []
    has_bias = bool(np.any(bias_full != 0.0))
    for c in range(8):
        bi, gi = c // 2, c % 2
        wq = np.concatenate(
            [w_qkv[:, d * DIM + gi * INNER_C: d * DIM + (gi + 1) * INNER_C]
             for d in range(3)], axis=1)
        bias = np.concatenate(
            [bias_full[d * DIM + gi * INNER_C: d * DIM + (gi + 1) * INNER_C]
             for d in range(3)])
        wo = w_out[gi * INNER_C:(gi + 1) * INNER_C, :]
        blob = np.empty(BLOB_WORDS, np.float32)
        blob[X_OFF:X_OFF + X_WORDS] = x[bi].reshape(-1)
        blob[WQ_OFF:WQ_OFF + WQ_WORDS] = _pack_bf16(wq)
        blob[WO_OFF:WO_OFF + WO_WORDS] = _pack_bf16(wo)
        blob[B_OFF:B_OFF + B_WORDS] = _pack_bf16(bias)
        in_maps.append({"blob": blob})
    return in_maps, has_bias


def gather_outputs(results):
    out = np.empty((4, N_TOK, DIM), np.float32)
    for bi in range(4):
        out[bi] = results[2 * bi]["out"] + results[2 * bi + 1]["out"]
    return out


def kernel(x, ln_gamma, ln_beta, w_qkv, w_out, **kw):
    from concourse.bass_utils import run_bass_kernel_spmd

    in_maps, has_bias = shard_inputs(x, ln_gamma, ln_beta, w_qkv, w_out)
    nc = get_nc(has_bias)
    res = run_bass_kernel_spmd(nc, in_maps, list(range(8)), **kw)
    _cache["last_results"] = res
    return gather_outputs(res.results)


# revision 53
# speedup vs baseline: 1.1092x; 1.1092x over previous
"""Trainium2 Bass kernel for nn_Attention (LN -> QKV -> MHA -> out-proj).

Sharding: 8 cores = 4 batches x 2 head-groups. Core c handles batch c//2,
heads (c%2)*8 .. (c%2)*8+8 (tensor-parallel split of w_qkv columns / w_out
rows). Each core emits a partial [2048, 1024] output; host sums the two
partials per batch.

Host-side precompute: LN gamma is folded into w_qkv rows (exact); beta
becomes a rank-1 bias row beta@w_qkv added on device only when nonzero.
All per-core inputs ship as ONE packed f32 dram tensor ("blob") because
per-exec dispatch overhead in this runtime grows ~60us per bound IO
tensor: [x (2048x1024 f32) | w_qkv' (1024x1536 bf16) | w_out (512x1024
bf16) | bias (1536 bf16)], bf16 regions bitcast from f32 words.

Device algorithm (per core), matmuls bf16 with f32 PSUM accumulation.
Schedule notes: GPSIMD cannot read PSUM on real TRN2 (CoreSim allows it),
so PSUM->SBUF copies go to ACT in phase 1 (idle there) and DVE elsewhere;
engine queues are in-order, so background work is placed where its
dependencies are already drained.
  1. Phase 1 per token tile: LayerNorm (f32 stats via bn_stats on DVE),
     cast bf16, PE-transpose to xnT [1024, 2048] (features on
     partitions) with ACT copying PSUM->SBUF; v = xn @ wv one group
     behind (ones column appended for the softmax denominator); all
     k-side projections kT = wk^T @ xnT plus the qp=0 half of qT; the
     whole first attention block (j=0, qp=0) weaves in as v/k tiles
     complete, so ACT's exp pipeline starts inside phase 1.
  2. Attention blocks (head pair j, query half qp): per kv tile t,
     scoresT = kT^T qT and exp(scale*scoresT) on ACT (no max
     subtraction -- scores are O(5) for these inputs) emit first, then
     one background unit, then the PREVIOUS tile's PV matmuls -- PE
     bursts from background work land ahead of a deferrable PV instead
     of stalling the exp feed (ACT is the in-block bottleneck at
     ~2.08us/tile vs PE ~1.7us). PV accumulates [out^T; denom] in PSUM.
     At block end only reciprocal (DVE) + raw copy (DVE) run, freeing
     PSUM for the next block; the normalize finish (ones-matmul
     broadcast + multiply into outn) defers into the next block's
     background slots (t0+2, t0+4, ...) where its chain has drained.
     Background slots also carry the q-side n=2,3 projections (only
     needed from qp=1 on) and, during qp=1, the out-projection of the
     qp=0 token half.
  3. out = sum_j outn[j]^T @ w_out[j], K=128 PSUM accumulation; DVE
     copies PSUM->SBUF; the qp=1 token half drains in a short tail.
"""

import os
import sys

sys.path.insert(0, "/opt/trn_rl_repo")
os.environ.setdefault("MYCRO_LOCAL_CACHE", "1")

import numpy as np
import ml_dtypes

N_TOK = 2048
DIM = 1024
HPC = 8          # heads per core
DH = 64          # head dim
INNER_C = HPC * DH  # 512 per-core inner width
NT = N_TOK // 128   # 16 token tiles
KF = DIM // 128     # 8 feature tiles
SCALE = DH ** -0.5

# blob layout (f32 word offsets)
X_WORDS = N_TOK * DIM                      # 2,097,152
WQ_WORDS = DIM * 3 * INNER_C // 2          # 786,432 (bf16 pairs)
WO_WORDS = INNER_C * DIM // 2              # 262,144
B_WORDS = 3 * INNER_C // 2                 # 768
X_OFF = 0
WQ_OFF = X_OFF + X_WORDS
WO_OFF = WQ_OFF + WQ_WORDS
B_OFF = WO_OFF + WO_WORDS
BLOB_WORDS = B_OFF + B_WORDS

_cache = {}


def _build_nc(has_bias):
    import concourse.bass as bass
    import concourse.mybir as mybir
    import concourse.tile as tile
    from concourse import bacc
    from concourse.masks import make_identity
    from contextlib import ExitStack

    f32 = mybir.dt.float32
    bf16 = mybir.dt.bfloat16
    nc = bacc.Bacc(None, target_bir_lowering=False)

    blob_d = nc.dram_tensor("blob", [BLOB_WORDS], f32, kind="ExternalInput")
    out_d = nc.dram_tensor("out", [N_TOK, DIM], f32, kind="ExternalOutput")

    def wq_slice(kc):
        w = blob_d[WQ_OFF + kc * 128 * (3 * INNER_C // 2):
                   WQ_OFF + (kc + 1) * 128 * (3 * INNER_C // 2)]
        return w.bitcast(bf16).rearrange("(p f) -> p f", p=128)

    def wo_slice(j):
        w = blob_d[WO_OFF + j * 128 * (DIM // 2):
                   WO_OFF + (j + 1) * 128 * (DIM // 2)]
        return w.bitcast(bf16).rearrange("(p f) -> p f", p=128)

    def x_slice(t):
        return blob_d[t * 128 * DIM:(t + 1) * 128 * DIM].rearrange(
            "(p f) -> p f", p=128)

    with tile.TileContext(nc) as tc, ExitStack() as ctx:
        consts = ctx.enter_context(tc.tile_pool(name="consts", bufs=1))
        weights = ctx.enter_context(tc.tile_pool(name="weights", bufs=1))
        persist = ctx.enter_context(tc.tile_pool(name="persist", bufs=1))
        work = ctx.enter_context(tc.tile_pool(name="work", bufs=3))
        nrm = ctx.enter_context(tc.tile_pool(name="nrm", bufs=1))
        stats = ctx.enter_context(tc.tile_pool(name="stats", bufs=4))
        mm_ps = ctx.enter_context(tc.tile_pool(name="mm_ps", bufs=2, space="PSUM"))
        pv_ps = ctx.enter_context(tc.tile_pool(name="pv_ps", bufs=2, space="PSUM"))

        ident = consts.tile([128, 128], bf16, tag="ident")
        make_identity(nc, ident)
        eps_t = consts.tile([128, 1], f32, tag="eps")
        nc.vector.memset(eps_t, 1e-5)
        ones_t = consts.tile([1, 64], bf16, tag="ones_t")
        nc.vector.memset(ones_t, 1.0)
        if has_bias:
            bias_sb = consts.tile([1, 3 * INNER_C], bf16, tag="bias")
            nc.sync.dma_start(
                out=bias_sb,
                in_=blob_d[B_OFF:B_OFF + B_WORDS].bitcast(bf16).rearrange(
                    "(p f) -> p f", p=1))
            ones_row = consts.tile([1, 512], bf16, tag="ones_row")
            nc.vector.memset(ones_row, 1.0)

        wo_sb = [weights.tile([128, DIM], bf16, tag=f"wo{j}", name=f"wo{j}")
                 for j in range(4)]

        qkT = [persist.tile([128, N_TOK], bf16, tag=f"qkT{m}", name=f"qkT{m}")
               for m in range(KF)]
        outn = [persist.tile([128, N_TOK], bf16, tag=f"outn{j}", name=f"outn{j}")
                for j in range(4)]
        v_aug = [persist.tile([128, HPC, DH + 1], bf16, tag=f"vaug{t}",
                              name=f"vaug{t}") for t in range(NT)]

        def out_proj(t):
            ps_o = mm_ps.tile([128, 1024], f32, tag="mm", name="ops")
            for c in range(2):
                for j in range(4):
                    nc.tensor.matmul(
                        ps_o[:, c * 512:(c + 1) * 512],
                        lhsT=outn[j][:, t * 128:(t + 1) * 128],
                        rhs=wo_sb[j][:, c * 512:(c + 1) * 512],
                        start=(j == 0), stop=(j == 3),
                    )
            osb = work.tile([128, DIM], f32, tag="osb", bufs=3)
            nc.vector.tensor_copy(out=osb, in_=ps_o)
            nc.sync.dma_start(out=out_d[t * 128:(t + 1) * 128, :], in_=osb)

        with tc.tile_pool(name="qkvw", bufs=1) as qkvw:
            wq_sb = [qkvw.tile([128, 3 * INNER_C], bf16, tag=f"wq{kc}",
                               name=f"wq{kc}") for kc in range(KF)]
            xnT = [qkvw.tile([128, N_TOK], bf16, tag=f"xnT{f}", name=f"xnT{f}")
                   for f in range(KF)]

            def add_bias(ps_view, col_lo, n):
                # ps_view[m, q] += bias[col_lo + m] for all q (rank-1).
                nc.tensor.matmul(
                    ps_view,
                    lhsT=bias_sb[:, col_lo:col_lo + 128],
                    rhs=ones_row[:, 0:n],
                    start=False, stop=True,
                )

            def v_group(t):
                vt = v_aug[t]
                nc.vector.memset(vt[:, :, DH:DH + 1], 1.0)
                ps = mm_ps.tile([128, 1024], f32, tag="mm", name="vps")
                for kc in range(KF):
                    nc.tensor.matmul(
                        ps[:, 0:512],
                        lhsT=xnT[kc][:, t * 128:(t + 1) * 128],
                        rhs=wq_sb[kc][:, 2 * INNER_C:3 * INNER_C],
                        start=(kc == 0),
                        stop=(kc == KF - 1 and not has_bias),
                    )
                if has_bias:
                    # v rows are tokens: ps[tok, m] += 1[tok] * bias_v[m]
                    nc.tensor.matmul(
                        ps[:, 0:512],
                        lhsT=ones_row[:, 0:128],
                        rhs=bias_sb[:, 2 * INNER_C:3 * INNER_C],
                        start=False, stop=True,
                    )
                nc.vector.tensor_copy(
                    out=vt[:, :, 0:DH],
                    in_=ps[:, 0:512].rearrange("p (h d) -> p h d", h=HPC),
                )

            def qk_group(m, n):
                ps = mm_ps.tile([128, 1024], f32, tag="mm", name="qkps")
                for kc in range(KF):
                    nc.tensor.matmul(
                        ps[:, 0:512],
                        lhsT=wq_sb[kc][:, m * 128:(m + 1) * 128],
                        rhs=xnT[kc][:, n * 512:(n + 1) * 512],
                        start=(kc == 0),
                        stop=(kc == KF - 1 and not has_bias),
                    )
                if has_bias:
                    add_bias(ps[:, 0:512], m * 128, 512)
                nc.scalar.copy(
                    out=qkT[m][:, n * 512:(n + 1) * 512], in_=ps[:, 0:512])

            class AttBlock:
                """Head pair j (heads 2j at partitions 0:64, 2j+1 at
                64:128), query half qp. step(t) consumes kv tile t;
                end() frees PSUM (reciprocal + raw copy) and returns two
                deferred finish closures (ones-matmul broadcast +
                normalize multiply into outn)."""

                def __init__(self, j, qp):
                    self.j, self.qp = j, qp
                    self.qlo = qp * 1024
                    self.pending = []
                    self.ps_pv = [pv_ps.tile([65, 1024], f32, tag="pv",
                                             name="pspv") for _ in range(2)]

                def step(self, t, bg=None, flush=True):
                    # scores+exp for t first, bg burst next, then the
                    # OLDEST pending step's PV: bg PE bursts land in front
                    # of a deferrable PV instead of delaying the exp feed.
                    # flush=False accumulates PVs (used to pre-run a
                    # block's exp feed while another still holds PSUM).
                    j, qlo = self.j, self.qlo
                    ets = []
                    for r in range(2):
                        q_ap = qkT[j][r * 64:(r + 1) * 64, :]
                        k_ap = qkT[4 + j][r * 64:(r + 1) * 64, :]
                        ps_s = mm_ps.tile([128, 1024], f32, tag="mm")
                        for half in range(2):
                            q0 = qlo + half * 512
                            nc.tensor.matmul(
                                ps_s[:, half * 512:(half + 1) * 512],
                                lhsT=k_ap[:, t * 128:(t + 1) * 128],
                                rhs=q_ap[:, q0:q0 + 512],
                                start=True, stop=True,
                            )
                        et = work.tile([128, 1024], bf16, tag="et", bufs=6)
                        nc.scalar.activation(
                            out=et, in_=ps_s[:, 0:1024],
                            func=mybir.ActivationFunctionType.Exp,
                            scale=SCALE,
                        )
                        ets.append(et)
                    if bg is not None:
                        bg()
                    if flush:
                        self._flush_pv()
                    self.pending.append((t, ets))

                def _flush_pv(self):
                    if not self.pending:
                        return
                    t, ets = self.pending.pop(0)
                    j = self.j
                    for r in range(2):
                        for c in range(2):
                            nc.tensor.matmul(
                                self.ps_pv[r][:, c * 512:(c + 1) * 512],
                                lhsT=v_aug[t][:, 2 * j + r, :],
                                rhs=ets[r][:, c * 512:(c + 1) * 512],
                                start=(t == 0), stop=(t == NT - 1),
                            )

                def end(self):
                    while self.pending:
                        self._flush_pv()
                    j, qlo = self.j, self.qlo
                    fins = []
                    for r in range(2):
                        rc = nrm.tile([1, 1024], f32, tag="rc", bufs=2)
                        nc.vector.reciprocal(out=rc, in_=self.ps_pv[r][64:65, :])
                        un = nrm.tile([64, 1024], bf16, tag="un", bufs=2)
                        nc.vector.tensor_copy(out=un, in_=self.ps_pv[r][0:64, :])

                        def fin(r=r, rc=rc, un=un):
                            rcb = nrm.tile([1, 1024], bf16, tag="rcb", bufs=2)
                            nc.gpsimd.tensor_copy(out=rcb, in_=rc)
                            for c in range(2):
                                bcps = mm_ps.tile([64, 512], f32, tag="mm",
                                                  name="bcps")
                                nc.tensor.matmul(
                                    bcps, lhsT=ones_t,
                                    rhs=rcb[:, c * 512:(c + 1) * 512],
                                    start=True, stop=True,
                                )
                                nc.vector.tensor_mul(
                                    out=outn[j][r * 64:(r + 1) * 64,
                                                qlo + c * 512:
                                                qlo + (c + 1) * 512],
                                    in0=un[0:64, c * 512:(c + 1) * 512],
                                    in1=bcps,
                                )
                        fins.append(fin)
                    return fins

            def run_block(blk, t0, bg, every=2):
                # bg closures fire one per slot at t = t0+2, t0+2+every,
                # ...: starting two steps in, the previous block's recip/
                # copy chain (which the first fin unit waits on) has
                # drained, so the fin's ones-matmul doesn't stall the
                # in-order PE queue.
                for t in range(t0, NT):
                    idx = (t - t0 - 2) // every
                    use = (bg[idx] if (t - t0) >= 2 and (t - t0 - 2) % every == 0
                           and idx < len(bg) else None)
                    blk.step(t, bg=use)
                return blk.end()

            # ------------ Phase 1: LayerNorm + transpose + qkv ------------
            with tc.tile_pool(name="ln", bufs=1) as lnp:
                for g in range(4):
                    for tt in range(4):
                        t = g * 4 + tt
                        xt = work.tile([128, DIM], f32, tag="xt", bufs=3)
                        nc.sync.dma_start(out=xt, in_=x_slice(t))
                        st = stats.tile([128, 2, 6], f32, tag="bn")
                        xr = xt.rearrange("p (s d) -> p s d", s=2)
                        for s in range(2):
                            nc.vector.bn_stats(out=st[:, s, :], in_=xr[:, s, :])
                        mv = stats.tile([128, 2], f32, tag="mv")
                        nc.vector.bn_aggr(out=mv, in_=st)
                        rsig = stats.tile([128, 1], f32, tag="rsig")
                        nc.scalar.activation(
                            out=rsig, in_=mv[:, 1:2],
                            func=mybir.ActivationFunctionType.Sqrt,
                            bias=eps_t, scale=1.0,
                        )
                        nc.vector.reciprocal(out=rsig, in_=rsig)
                        xn = lnp.tile([128, DIM], bf16, tag=f"xn{tt}",
                                      name=f"xn{tt}", bufs=2)
                        nc.vector.tensor_scalar(
                            out=xn, in0=xt, scalar1=mv[:, 0:1], scalar2=rsig,
                            op0=mybir.AluOpType.subtract,
                            op1=mybir.AluOpType.mult,
                        )
                        # PE transpose into one PSUM tile; ACT (idle in
                        # phase 1) copies it out to xnT. v_group runs one
                        # group behind so w_qkv (loaded after the first x
                        # tiles) is resident when v_group(0) issues.
                        trps = mm_ps.tile([128, KF, 128], bf16, tag="mm",
                                          name="trps")
                        for f in range(KF):
                            nc.tensor.transpose(
                                out=trps[:, f, :],
                                in_=xn[:, f * 128:(f + 1) * 128],
                                identity=ident,
                            )
                        for f in range(KF):
                            nc.scalar.copy(
                                out=xnT[f][:, t * 128:(t + 1) * 128],
                                in_=trps[:, f, :],
                            )
                        if t >= 4:
                            v_group(t - 4)
                    if g == 0:
                        for kc in range(KF):
                            nc.sync.dma_start(out=wq_sb[kc], in_=wq_slice(kc))
                    if g == 1:
                        for j in range(4):
                            nc.sync.dma_start(out=wo_sb[j], in_=wo_slice(j))
                    # Chunks n=0,1 of everything build in phase 1, plus
                    # m=4 (head pair 0's k side, read by the woven att0 at
                    # t=8..15). Remaining n=2,3 chunks ride the attention
                    # bg slots: each block's own k-side chunk fires two
                    # steps before its first kv reader; q-side chunks are
                    # only read by qp=1 blocks.
                    for m in range(KF) if g < 2 else (4,):
                        qk_group(m, g)
                    if g == 3:
                        for t4 in range(NT - 4, NT):
                            v_group(t4)
                    # weave the first attention block's kv steps into
                    # phase 1 as their q/k/v dependencies complete (kv tile
                    # t needs v_aug[t] and kT chunk t//4 from group t//4;
                    # its q columns 0:1024 need groups 0-1).
                    if g == 2:
                        att0 = AttBlock(0, 0)
                        for t in range(8):
                            att0.step(t)
                    elif g == 3:
                        for t in range(8, NT):
                            att0.step(t)

            # block order: (j,qp=0) j=0..3 then (j,qp=1); each block carries
            # the previous block's deferred normalize finish, the deferred
            # q-side n=2,3 projections, and (during qp=1) the out-proj of
            # the qp=0 token half.
            def qk_unit(m, n):
                return lambda: qk_group(m, n)

            fins = att0.end()
            for j in range(1, 4):
                # order matters: this block's k-side chunk n must fire
                # before its first reader at kv step t=4n.
                extra = [qk_unit(4 + j, 2), qk_unit(j - 1, 2),
                         qk_unit(4 + j, 3), qk_unit(j - 1, 3)]
                fins = run_block(AttBlock(j, 0), 0, fins + extra)
            for j in range(4):
                extra = [qk_unit(3, 2 + j)] if j < 2 else []
                ops = [(lambda t=2 * j + tt: out_proj(t)) for tt in range(2)]
                fins = run_block(AttBlock(j, 1), 0, fins + extra + ops)

        # ------------ tail: deferred finish + remaining out-projection ----
        for fin in fins:
            fin()
        for t in range(8, NT):
            out_proj(t)

    nc.compile()
    return nc


def get_nc(has_bias=False):
    key = ("nc", has_bias)
    if key not in _cache:
        _cache[key] = _build_nc(has_bias)
    return _cache[key]


def _pack_bf16(a):
    """bf16 array -> f32 word view (pairs little-endian)."""
    b = np.ascontiguousarray(a.astype(ml_dtypes.bfloat16))
    return b.reshape(-1).view(np.float32)


def shard_inputs(x, ln_gamma, ln_beta, w_qkv, w_out):
    """Returns (per-core input maps, has_bias) for 8 cores."""
    x = np.asarray(x, np.float32)
    g = np.asarray(ln_gamma, np.float32)
    b = np.asarray(ln_beta, np.float32)
    w0 = np.asarray(w_qkv, np.float32)
    bias_full = b @ w0                 # beta through the projection
    w_qkv = w0 * g[:, None]            # fold LN gamma into rows
    w_out = np.asarray(w_out, np.float32)
    in_maps = []
    has_bias = bool(np.any(bias_full != 0.0))
    for c in range(8):
        bi, gi = c // 2, c % 2
        wq = np.concatenate(
            [w_qkv[:, d * DIM + gi * INNER_C: d * DIM + (gi + 1) * INNER_C]
             for d in range(3)], axis=1)
        bias = np.concatenate(
            [bias_full[d * DIM + gi * INNER_C: d * DIM + (gi + 1) * INNER_C]
             for d in range(3)])
        wo = w_out[gi * INNER_C:(gi + 1) * INNER_C, :]
        blob = np.empty(BLOB_WORDS, np.float32)
        blob[X_OFF:X_OFF + X_WORDS] = x[bi].reshape(-1)
        blob[WQ_OFF:WQ_OFF + WQ_WORDS] = _pack_bf16(wq)
        blob[WO_OFF:WO_OFF + WO_WORDS] = _pack_bf16(wo)
        blob[B_OFF:B_OFF + B_WORDS] = _pack_bf16(bias)
        in_maps.append({"blob": blob})
    return in_maps, has_bias


def gather_outputs(results):
    out = np.empty((4, N_TOK, DIM), np.float32)
    for bi in range(4):
        out[bi] = results[2 * bi]["out"] + results[2 * bi + 1]["out"]
    return out


def kernel(x, ln_gamma, ln_beta, w_qkv, w_out, **kw):
    from concourse.bass_utils import run_bass_kernel_spmd

    in_maps, has_bias = shard_inputs(x, ln_gamma, ln_beta, w_qkv, w_out)
    nc = get_nc(has_bias)
    res = run_bass_kernel_spmd(nc, in_maps, list(range(8)), **kw)
    _cache["last_results"] = res
    return gather_outputs(res.results)


# revision 59
# speedup vs baseline: 1.1289x; 1.0178x over previous
"""Trainium2 Bass kernel for nn_Attention (LN -> QKV -> MHA -> out-proj).

Sharding: 8 cores = 4 batches x 2 head-groups. Core c handles batch c//2,
heads (c%2)*8 .. (c%2)*8+8 (tensor-parallel split of w_qkv columns / w_out
rows). Each core emits a partial [2048, 1024] output; host sums the two
partials per batch.

Host-side precompute: LN gamma is folded into w_qkv rows (exact); beta
becomes a rank-1 bias row beta@w_qkv added on device only when nonzero.
All per-core inputs ship as ONE packed f32 dram tensor ("blob") because
per-exec dispatch overhead in this runtime grows ~60us per bound IO
tensor: [x (2048x1024 f32) | w_qkv' (1024x1536 bf16) | w_out (512x1024
bf16) | bias (1536 bf16)], bf16 regions bitcast from f32 words.

Device algorithm (per core), matmuls bf16 with f32 PSUM accumulation.
Schedule notes: GPSIMD cannot read PSUM on real TRN2 (CoreSim allows it),
so PSUM->SBUF copies go to ACT in phase 1 (idle there) and DVE elsewhere;
engine queues are in-order, so background work is placed where its
dependencies are already drained.
  1. Phase 1 per token tile: LayerNorm (f32 stats via bn_stats on DVE),
     cast bf16, PE-transpose to xnT [1024, 2048] (features on
     partitions) with ACT copying PSUM->SBUF; v = xn @ wv one group
     behind (ones column appended for the softmax denominator); all
     k-side projections kT = wk^T @ xnT plus the qp=0 half of qT; the
     whole first attention block (j=0, qp=0) weaves in as v/k tiles
     complete, so ACT's exp pipeline starts inside phase 1.
  2. Attention blocks (head pair j, query half qp): per kv tile t,
     scoresT = kT^T qT and exp(scale*scoresT) on ACT (no max
     subtraction -- scores are O(5) for these inputs) emit first, then
     one background unit, then the PREVIOUS tile's PV matmuls -- PE
     bursts from background work land ahead of a deferrable PV instead
     of stalling the exp feed (ACT is the in-block bottleneck at
     ~2.08us/tile vs PE ~1.7us). PV accumulates [out^T; denom] in PSUM.
     At block end only reciprocal (DVE) + raw copy (DVE) run, freeing
     PSUM for the next block; the normalize finish (ones-matmul
     broadcast + multiply into outn) defers into the next block's
     background slots (t0+2, t0+4, ...) where its chain has drained.
     Background slots also carry the q-side n=2,3 projections (only
     needed from qp=1 on) and, during qp=1, the out-projection of the
     qp=0 token half.
  3. out = sum_j outn[j]^T @ w_out[j], K=128 PSUM accumulation; DVE
     copies PSUM->SBUF; the qp=1 token half drains in a short tail.
"""

import os
import sys

sys.path.insert(0, "/opt/trn_rl_repo")
os.environ.setdefault("MYCRO_LOCAL_CACHE", "1")

import numpy as np
import ml_dtypes

N_TOK = 2048
DIM = 1024
HPC = 8          # heads per core
DH = 64          # head dim
INNER_C = HPC * DH  # 512 per-core inner width
NT = N_TOK // 128   # 16 token tiles
KF = DIM // 128     # 8 feature tiles
SCALE = DH ** -0.5

# blob layout (f32 word offsets)
X_WORDS = N_TOK * DIM                      # 2,097,152
WQ_WORDS = DIM * 3 * INNER_C // 2          # 786,432 (bf16 pairs)
WO_WORDS = INNER_C * DIM // 2              # 262,144
B_WORDS = 3 * INNER_C // 2                 # 768
X_OFF = 0
WQ_OFF = X_OFF + X_WORDS
WO_OFF = WQ_OFF + WQ_WORDS
B_OFF = WO_OFF + WO_WORDS
BLOB_WORDS = B_OFF + B_WORDS

_cache = {}


def _build_nc(has_bias):
    import concourse.bass as bass
    import concourse.mybir as mybir
    import concourse.tile as tile
    from concourse import bacc
    from concourse.masks import make_identity
    from contextlib import ExitStack

    f32 = mybir.dt.float32
    bf16 = mybir.dt.bfloat16
    nc = bacc.Bacc(None, target_bir_lowering=False)

    blob_d = nc.dram_tensor("blob", [BLOB_WORDS], f32, kind="ExternalInput")
    out_d = nc.dram_tensor("out", [N_TOK, DIM], f32, kind="ExternalOutput")

    def wq_slice(kc):
        w = blob_d[WQ_OFF + kc * 128 * (3 * INNER_C // 2):
                   WQ_OFF + (kc + 1) * 128 * (3 * INNER_C // 2)]
        return w.bitcast(bf16).rearrange("(p f) -> p f", p=128)

    def wo_slice(j):
        w = blob_d[WO_OFF + j * 128 * (DIM // 2):
                   WO_OFF + (j + 1) * 128 * (DIM // 2)]
        return w.bitcast(bf16).rearrange("(p f) -> p f", p=128)

    def x_slice(t):
        return blob_d[t * 128 * DIM:(t + 1) * 128 * DIM].rearrange(
            "(p f) -> p f", p=128)

    with tile.TileContext(nc) as tc, ExitStack() as ctx:
        consts = ctx.enter_context(tc.tile_pool(name="consts", bufs=1))
        weights = ctx.enter_context(tc.tile_pool(name="weights", bufs=1))
        persist = ctx.enter_context(tc.tile_pool(name="persist", bufs=1))
        work = ctx.enter_context(tc.tile_pool(name="work", bufs=3))
        nrm = ctx.enter_context(tc.tile_pool(name="nrm", bufs=1))
        stats = ctx.enter_context(tc.tile_pool(name="stats", bufs=4))
        mm_ps = ctx.enter_context(tc.tile_pool(name="mm_ps", bufs=2, space="PSUM"))
        pv_ps = ctx.enter_context(tc.tile_pool(name="pv_ps", bufs=2, space="PSUM"))

        ident = consts.tile([128, 128], bf16, tag="ident")
        make_identity(nc, ident)
        eps_t = consts.tile([128, 1], f32, tag="eps")
        nc.vector.memset(eps_t, 1e-5)
        ones_t = consts.tile([1, 64], bf16, tag="ones_t")
        nc.vector.memset(ones_t, 1.0)
        if has_bias:
            bias_sb = consts.tile([1, 3 * INNER_C], bf16, tag="bias")
            nc.sync.dma_start(
                out=bias_sb,
                in_=blob_d[B_OFF:B_OFF + B_WORDS].bitcast(bf16).rearrange(
                    "(p f) -> p f", p=1))
            ones_row = consts.tile([1, 512], bf16, tag="ones_row")
            nc.vector.memset(ones_row, 1.0)

        wo_sb = [weights.tile([128, DIM], bf16, tag=f"wo{j}", name=f"wo{j}")
                 for j in range(4)]

        qkT = [persist.tile([128, N_TOK], bf16, tag=f"qkT{m}", name=f"qkT{m}")
               for m in range(KF)]
        outn = [persist.tile([128, N_TOK], bf16, tag=f"outn{j}", name=f"outn{j}")
                for j in range(4)]
        v_aug = [persist.tile([128, HPC, DH + 1], bf16, tag=f"vaug{t}",
                              name=f"vaug{t}") for t in range(NT)]

        def out_proj(t):
            ps_o = mm_ps.tile([128, 1024], f32, tag="mm", name="ops")
            for c in range(2):
                for j in range(4):
                    nc.tensor.matmul(
                        ps_o[:, c * 512:(c + 1) * 512],
                        lhsT=outn[j][:, t * 128:(t + 1) * 128],
                        rhs=wo_sb[j][:, c * 512:(c + 1) * 512],
                        start=(j == 0), stop=(j == 3),
                    )
            osb = work.tile([128, DIM], f32, tag="osb", bufs=3)
            nc.vector.tensor_copy(out=osb, in_=ps_o)
            nc.sync.dma_start(out=out_d[t * 128:(t + 1) * 128, :], in_=osb)

        with tc.tile_pool(name="qkvw", bufs=1) as qkvw:
            wq_sb = [qkvw.tile([128, 3 * INNER_C], bf16, tag=f"wq{kc}",
                               name=f"wq{kc}") for kc in range(KF)]
            xnT = [qkvw.tile([128, N_TOK], bf16, tag=f"xnT{f}", name=f"xnT{f}")
                   for f in range(KF)]

            def add_bias(ps_view, col_lo, n):
                # ps_view[m, q] += bias[col_lo + m] for all q (rank-1).
                nc.tensor.matmul(
                    ps_view,
                    lhsT=bias_sb[:, col_lo:col_lo + 128],
                    rhs=ones_row[:, 0:n],
                    start=False, stop=True,
                )

            def v_group(t):
                vt = v_aug[t]
                nc.vector.memset(vt[:, :, DH:DH + 1], 1.0)
                ps = mm_ps.tile([128, 1024], f32, tag="mm", name="vps")
                for kc in range(KF):
                    nc.tensor.matmul(
                        ps[:, 0:512],
                        lhsT=xnT[kc][:, t * 128:(t + 1) * 128],
                        rhs=wq_sb[kc][:, 2 * INNER_C:3 * INNER_C],
                        start=(kc == 0),
                        stop=(kc == KF - 1 and not has_bias),
                    )
                if has_bias:
                    # v rows are tokens: ps[tok, m] += 1[tok] * bias_v[m]
                    nc.tensor.matmul(
                        ps[:, 0:512],
                        lhsT=ones_row[:, 0:128],
                        rhs=bias_sb[:, 2 * INNER_C:3 * INNER_C],
                        start=False, stop=True,
                    )
                nc.vector.tensor_copy(
                    out=vt[:, :, 0:DH],
                    in_=ps[:, 0:512].rearrange("p (h d) -> p h d", h=HPC),
                )

            def qk_group(m, n):
                ps = mm_ps.tile([128, 1024], f32, tag="mm", name="qkps")
                for kc in range(KF):
                    nc.tensor.matmul(
                        ps[:, 0:512],
                        lhsT=wq_sb[kc][:, m * 128:(m + 1) * 128],
                        rhs=xnT[kc][:, n * 512:(n + 1) * 512],
                        start=(kc == 0),
                        stop=(kc == KF - 1 and not has_bias),
                    )
                if has_bias:
                    add_bias(ps[:, 0:512], m * 128, 512)
                nc.scalar.copy(
                    out=qkT[m][:, n * 512:(n + 1) * 512], in_=ps[:, 0:512])

            class AttBlock:
                """Head pair j (heads 2j at partitions 0:64, 2j+1 at
                64:128), query half qp. step(t) consumes kv tile t;
                end() frees PSUM (reciprocal + raw copy) and returns two
                deferred finish closures (ones-matmul broadcast +
                normalize multiply into outn)."""

                def __init__(self, j, qp):
                    self.j, self.qp = j, qp
                    self.qlo = qp * 1024
                    self.pending = []
                    self.ps_pv = [pv_ps.tile([65, 1024], f32, tag="pv",
                                             name="pspv") for _ in range(2)]

                def step(self, t, bg=None, flush=True):
                    # scores+exp for t first, bg burst next, then the
                    # OLDEST pending step's PV: bg PE bursts land in front
                    # of a deferrable PV instead of delaying the exp feed.
                    # flush=False accumulates PVs (used to pre-run a
                    # block's exp feed while another still holds PSUM).
                    j, qlo = self.j, self.qlo
                    ets = []
                    for r in range(2):
                        q_ap = qkT[j][r * 64:(r + 1) * 64, :]
                        k_ap = qkT[4 + j][r * 64:(r + 1) * 64, :]
                        ps_s = mm_ps.tile([128, 1024], f32, tag="mm")
                        for half in range(2):
                            q0 = qlo + half * 512
                            nc.tensor.matmul(
                                ps_s[:, half * 512:(half + 1) * 512],
                                lhsT=k_ap[:, t * 128:(t + 1) * 128],
                                rhs=q_ap[:, q0:q0 + 512],
                                start=True, stop=True,
                            )
                        et = work.tile([128, 1024], bf16, tag="et", bufs=6)
                        nc.scalar.activation(
                            out=et, in_=ps_s[:, 0:1024],
                            func=mybir.ActivationFunctionType.Exp,
                            scale=SCALE,
                        )
                        ets.append(et)
                    if bg is not None:
                        bg()
                    if flush:
                        self._flush_pv()
                    self.pending.append((t, ets))

                def _flush_pv(self):
                    if not self.pending:
                        return
                    t, ets = self.pending.pop(0)
                    j = self.j
                    for r in range(2):
                        for c in range(2):
                            nc.tensor.matmul(
                                self.ps_pv[r][:, c * 512:(c + 1) * 512],
                                lhsT=v_aug[t][:, 2 * j + r, :],
                                rhs=ets[r][:, c * 512:(c + 1) * 512],
                                start=(t == 0), stop=(t == NT - 1),
                            )

                def end(self):
                    while self.pending:
                        self._flush_pv()
                    j, qlo = self.j, self.qlo
                    fins = []
                    for r in range(2):
                        rc = nrm.tile([1, 1024], f32, tag="rc", bufs=2)
                        nc.vector.reciprocal(out=rc, in_=self.ps_pv[r][64:65, :])
                        un = nrm.tile([64, 1024], bf16, tag="un", bufs=2)
                        nc.vector.tensor_copy(out=un, in_=self.ps_pv[r][0:64, :])

                        def fin(r=r, rc=rc, un=un):
                            rcb = nrm.tile([1, 1024], bf16, tag="rcb", bufs=2)
                            nc.gpsimd.tensor_copy(out=rcb, in_=rc)
                            for c in range(2):
                                bcps = mm_ps.tile([64, 512], f32, tag="mm",
                                                  name="bcps")
                                nc.tensor.matmul(
                                    bcps, lhsT=ones_t,
                                    rhs=rcb[:, c * 512:(c + 1) * 512],
                                    start=True, stop=True,
                                )
                                nc.vector.tensor_mul(
                                    out=outn[j][r * 64:(r + 1) * 64,
                                                qlo + c * 512:
                                                qlo + (c + 1) * 512],
                                    in0=un[0:64, c * 512:(c + 1) * 512],
                                    in1=bcps,
                                )
                        fins.append(fin)
                    return fins

            def run_block(blk, t0, bg, every=2):
                # bg closures fire one per slot at t = t0+2, t0+2+every,
                # ...: starting two steps in, the previous block's recip/
                # copy chain (which the first fin unit waits on) has
                # drained, so the fin's ones-matmul doesn't stall the
                # in-order PE queue.
                for t in range(t0, NT):
                    idx = (t - t0 - 2) // every
                    use = (bg[idx] if (t - t0) >= 2 and (t - t0 - 2) % every == 0
                           and idx < len(bg) else None)
                    blk.step(t, bg=use)
                return blk.end()

            # ------------ Phase 1: LayerNorm + transpose + qkv ------------
            with tc.tile_pool(name="ln", bufs=1) as lnp:
                for g in range(4):
                    for tt in range(4):
                        t = g * 4 + tt
                        xt = work.tile([128, DIM], f32, tag="xt", bufs=3)
                        nc.sync.dma_start(out=xt, in_=x_slice(t))
                        st = stats.tile([128, 2, 6], f32, tag="bn")
                        xr = xt.rearrange("p (s d) -> p s d", s=2)
                        for s in range(2):
                            nc.vector.bn_stats(out=st[:, s, :], in_=xr[:, s, :])
                        mv = stats.tile([128, 2], f32, tag="mv")
                        nc.vector.bn_aggr(out=mv, in_=st)
                        rsig = stats.tile([128, 1], f32, tag="rsig")
                        nc.scalar.activation(
                            out=rsig, in_=mv[:, 1:2],
                            func=mybir.ActivationFunctionType.Sqrt,
                            bias=eps_t, scale=1.0,
                        )
                        nc.vector.reciprocal(out=rsig, in_=rsig)
                        xn = lnp.tile([128, DIM], bf16, tag=f"xn{tt}",
                                      name=f"xn{tt}", bufs=2)
                        nc.vector.tensor_scalar(
                            out=xn, in0=xt, scalar1=mv[:, 0:1], scalar2=rsig,
                            op0=mybir.AluOpType.subtract,
                            op1=mybir.AluOpType.mult,
                        )
                        # PE transpose into one PSUM tile; ACT (idle in
                        # phase 1) copies it out to xnT. v_group runs one
                        # group behind so w_qkv (loaded after the first x
                        # tiles) is resident when v_group(0) issues.
                        trps = mm_ps.tile([128, KF, 128], bf16, tag="mm",
                                          name="trps")
                        for f in range(KF):
                            nc.tensor.transpose(
                                out=trps[:, f, :],
                                in_=xn[:, f * 128:(f + 1) * 128],
                                identity=ident,
                            )
                        for f in range(KF):
                            nc.scalar.copy(
                                out=xnT[f][:, t * 128:(t + 1) * 128],
                                in_=trps[:, f, :],
                            )
                        if t >= 4:
                            v_group(t - 4)
                    if g == 0:
                        for kc in range(KF):
                            nc.sync.dma_start(out=wq_sb[kc], in_=wq_slice(kc))
                    if g == 1:
                        for j in range(4):
                            nc.sync.dma_start(out=wo_sb[j], in_=wo_slice(j))
                    # Chunks n=0,1 of everything build in phase 1, plus
                    # m=4 (head pair 0's k side, read by the woven att0 at
                    # t=8..15). Remaining n=2,3 chunks ride the attention
                    # bg slots: each block's own k-side chunk fires two
                    # steps before its first kv reader; q-side chunks are
                    # only read by qp=1 blocks.
                    for m in range(KF) if g < 2 else (4,):
                        qk_group(m, g)
                    if g == 3:
                        for t4 in range(NT - 4, NT):
                            v_group(t4)
                    # weave the first attention block's kv steps into
                    # phase 1 as their q/k/v dependencies complete (kv tile
                    # t needs v_aug[t] and kT chunk t//4 from group t//4;
                    # its q columns 0:1024 need groups 0-1).
                    if g == 2:
                        att0 = AttBlock(0, 0)
                        for t in range(8):
                            att0.step(t)
                    elif g == 3:
                        for t in range(8, NT):
                            att0.step(t)

            # block order: (j,qp=0) j=0..3 then (j,qp=1); each block carries
            # the previous block's deferred normalize finish, the deferred
            # q-side n=2,3 projections, and (during qp=1) the out-proj of
            # the qp=0 token half.
            def qk_unit(m, n):
                return lambda: qk_group(m, n)

            fins = att0.end()
            for j in range(1, 4):
                # order matters: this block's k-side chunk n must fire
                # before its first reader at kv step t=4n.
                extra = [qk_unit(4 + j, 2), qk_unit(j - 1, 2),
                         qk_unit(4 + j, 3), qk_unit(j - 1, 3)]
                fins = run_block(AttBlock(j, 0), 0, fins + extra)
            for j in range(4):
                extra = [qk_unit(3, 2 + j)] if j < 2 else []
                ops = [(lambda t=2 * j + tt: out_proj(t)) for tt in range(2)]
                fins = run_block(AttBlock(j, 1), 0, fins + extra + ops)

        # ------------ tail: deferred finish + remaining out-projection ----
        for fin in fins:
            fin()
        for t in range(8, NT):
            out_proj(t)

    nc.compile()
    return nc


def get_nc(has_bias=False):
    key = ("nc", has_bias)
    if key not in _cache:
        _cache[key] = _build_nc(has_bias)
    return _cache[key]


def _pack_bf16(a):
    """bf16 array -> f32 word view (pairs little-endian)."""
    b = np.ascontiguousarray(a.astype(ml_dtypes.bfloat16))
    return b.reshape(-1).view(np.float32)


def shard_inputs(x, ln_gamma, ln_beta, w_qkv, w_out):
    """Returns (per-core input maps, has_bias) for 8 cores."""
    x = np.asarray(x, np.float32)
    g = np.asarray(ln_gamma, np.float32)
    b = np.asarray(ln_beta, np.float32)
    w0 = np.asarray(w_qkv, np.float32)
    bias_full = b @ w0                 # beta through the projection
    w_qkv = w0 * g[:, None]            # fold LN gamma into rows
    w_out = np.asarray(w_out, np.float32)
    in_maps = []
    has_bias = bool(np.any(bias_full != 0.0))
    for c in range(8):
        bi, gi = c // 2, c % 2
        wq = np.concatenate(
            [w_qkv[:, d * DIM + gi * INNER_C: d * DIM + (gi + 1) * INNER_C]
             for d in range(3)], axis=1)
        bias = np.concatenate(
            [bias_full[d * DIM + gi * INNER_C: d * DIM + (gi + 1) * INNER_C]
             for d in range(3)])
        wo = w_out[gi * INNER_C:(gi + 1) * INNER_C, :]
        blob = np.empty(BLOB_WORDS, np.float32)
        blob[X_OFF:X_OFF + X_WORDS] = x[bi].reshape(-1)
        blob[WQ_OFF:WQ_OFF + WQ_WORDS] = _pack_bf16(wq)
        blob[WO_OFF:WO_OFF + WO_WORDS] = _pack_bf16(wo)
        blob[B_OFF:B_OFF + B_WORDS] = _pack_bf16(bias)
        in_maps.append({"blob": blob})
    return in_maps, has_bias


def gather_outputs(results):
    out = np.empty((4, N_TOK, DIM), np.float32)
    for bi in range(4):
        out[bi] = results[2 * bi]["out"] + results[2 * bi + 1]["out"]
    return out


def kernel(x, ln_gamma, ln_beta, w_qkv, w_out, **kw):
    from concourse.bass_utils import run_bass_kernel_spmd

    in_maps, has_bias = shard_inputs(x, ln_gamma, ln_beta, w_qkv, w_out)
    nc = get_nc(has_bias)
    res = run_bass_kernel_spmd(nc, in_maps, list(range(8)), **kw)
    _cache["last_results"] = res
    return gather_outputs(res.results)
